# revision 4
# baseline (speedup 1.0000x reference)
"""Trainium2 Bass kernel for a non-selective (LTI) SSM.

Reference computation (per batch b, channel d):
    h_l = A @ h_{l-1} + Bvec * u[b, d, l]        (h in R^N, A = diag(a))
    y[b, d, l] = Cvec . h_l

Because the system is linear time-invariant and A is diagonal, the scan
collapses into a causal convolution with taps k_j = sum_i C_i a_i^j B_i.
We compute it with a chunked algorithm (chunk length Q = 128):

    y_intra[c] = TQ   @ u[c]      TQ lower-tri Toeplitz from k[0..Q-1]
    s[c]       = P    @ u[c]      end-of-chunk state from in-chunk inputs
    h[c]       = a^Q * h[c-1] + s[c]     (cheap 16-step scan, diagonal)
    y[c]       = y_intra[c] + W @ h[c-1] W[t, i] = C_i a_i^(t+1)

Everything is matmuls on the PE array except the 16-step carry scan.

Sharding: data-parallel over d_model (512 / 8 cores = 64 channels/core);
each core processes S = 4 batches x 64 channels = 256 sequences.
"""

import sys

sys.path.insert(0, "/opt/trn_rl_repo")

import numpy as np

import concourse.bass as bass
import concourse.mybir as mybir
import concourse.tile as tile
from concourse import bacc
from concourse.bass_utils import run_bass_kernel_spmd

N_CORES = 8
BATCH = 4
D_MODEL = 512
SEQ_LEN = 2048
N_STATE = 64
Q = 128                       # chunk length == partition dim
NCHUNK = SEQ_LEN // Q         # 16
D_PER_CORE = D_MODEL // N_CORES  # 64
S = BATCH * D_PER_CORE        # 256 sequences per core
GRP = 4                       # chunks per input DMA group
F32 = mybir.dt.float32


def build_program(mm_dtype=F32):
    """Build the per-core Bass program (identical on all 8 cores)."""
    nc = bacc.Bacc(None, target_bir_lowering=False)

    u_d = nc.declare_dram_parameter("u", [NCHUNK, Q, S], F32, isOutput=False)
    tqt_d = nc.declare_dram_parameter("tqt", [Q, Q], F32, isOutput=False)
    pt_d = nc.declare_dram_parameter("pt", [Q, N_STATE], F32, isOutput=False)
    wt_d = nc.declare_dram_parameter("wt", [N_STATE, Q], F32, isOutput=False)
    aq_d = nc.declare_dram_parameter("aq", [N_STATE, 1], F32, isOutput=False)
    y_d = nc.declare_dram_parameter("y", [NCHUNK, Q, S], F32, isOutput=True)

    mm = lambda ap: ap if mm_dtype == F32 else ap.bitcast(mm_dtype)

    with tile.TileContext(nc) as tc:
        with (
            tc.tile_pool(name="consts", bufs=1) as cpool,
            tc.tile_pool(name="upool", bufs=NCHUNK // GRP) as upool,
            tc.tile_pool(name="hpool", bufs=NCHUNK) as hpool,
            tc.tile_pool(name="ypool", bufs=4) as ypool,
            tc.tile_pool(name="ps_s", bufs=4, space="PSUM") as ps_s,
            tc.tile_pool(name="ps_y", bufs=3, space="PSUM") as ps_y,
        ):
            tqt = cpool.tile([Q, Q], F32)
            nc.sync.dma_start(out=tqt[:], in_=tqt_d[:])
            pt = cpool.tile([Q, N_STATE], F32)
            nc.sync.dma_start(out=pt[:], in_=pt_d[:])
            wt = cpool.tile([N_STATE, Q], F32)
            nc.sync.dma_start(out=wt[:], in_=wt_d[:])
            aq = cpool.tile([N_STATE, 1], F32)
            nc.sync.dma_start(out=aq[:], in_=aq_d[:])

            # u chunks: SBUF layout (t, c_local, s); HBM is (c, t, s)
            u_tiles = []
            for g in range(NCHUNK // GRP):
                ug = upool.tile([Q, GRP, S], F32, name="ug", tag="ug")
                nc.sync.dma_start(
                    out=ug[:],
                    in_=u_d[g * GRP:(g + 1) * GRP].transpose([1, 0, 2]),
                )
                for jj in range(GRP):
                    u_tiles.append(ug[:, jj, :])

            h_prev = None
            for c in range(NCHUNK):
                # end-of-chunk state contribution s[c] = P @ u[c]
                ps = ps_s.tile([N_STATE, S], F32, name="ps", tag="ps")
                nc.tensor.matmul(
                    ps[:], mm(pt[:]), mm(u_tiles[c]), start=True, stop=True
                )
                # carry scan h[c] = a^Q * h[c-1] + s[c]
                h = hpool.tile([N_STATE, S], F32, name="h", tag="h")
                if c == 0:
                    nc.vector.tensor_copy(out=h[:], in_=ps[:])
                else:
                    nc.vector.scalar_tensor_tensor(
                        out=h[:],
                        in0=h_prev[:],
                        scalar=aq[:],
                        in1=ps[:],
                        op0=mybir.AluOpType.mult,
                        op1=mybir.AluOpType.add,
                    )
                # y[c] = TQ @ u[c] (+ W @ h[c-1])
                py = ps_y.tile([Q, S], F32, name="py", tag="py")
                nc.tensor.matmul(
                    py[:], mm(tqt[:]), mm(u_tiles[c]), start=True, stop=(c == 0)
                )
                if c > 0:
                    nc.tensor.matmul(
                        py[:], mm(wt[:]), mm(h_prev[:]), start=False, stop=True
                    )
                yt = ypool.tile([Q, S], F32, name="yt", tag="yt")
                nc.any.tensor_copy(out=yt[:], in_=py[:])
                nc.sync.dma_start(out=y_d[c], in_=yt[:])
                h_prev = h

    nc.compile()
    return nc


def make_params(A, Bvec, Cvec):
    """Host-side precompute of the filter matrices (float64 -> float32)."""
    a = np.diag(np.asarray(A, np.float64))
    B64 = np.asarray(Bvec, np.float64)
    C64 = np.asarray(Cvec, np.float64)
    j = np.arange(Q)
    k = (a[None, :] ** j[:, None]) @ (C64 * B64)        # taps k[0..Q-1]
    TQt = np.zeros((Q, Q), np.float64)                  # TQt[t, jc] = k[jc-t]
    for t in range(Q):
        TQt[t, t:] = k[: Q - t]
    PT = (a[None, :] ** (Q - 1 - j)[:, None]) * B64[None, :]   # (Q, N)
    WT = C64[:, None] * (a[:, None] ** (j[None, :] + 1))       # (N, Q)
    aq = (a ** Q)[:, None]                                      # (N, 1)
    f32c = lambda x: np.ascontiguousarray(x, np.float32)
    return f32c(TQt), f32c(PT), f32c(WT), f32c(aq)


_prog_cache = {}


def get_program(mm_dtype=F32):
    key = str(mm_dtype)
    if key not in _prog_cache:
        _prog_cache[key] = build_program(mm_dtype)
    return _prog_cache[key]


def shard_inputs(u, A, Bvec, Cvec):
    """FULL inputs -> per-core in_maps."""
    TQt, PT, WT, aq = make_params(A, Bvec, Cvec)
    u = np.asarray(u, np.float32)
    in_maps = []
    for core in range(N_CORES):
        us = u[:, core * D_PER_CORE:(core + 1) * D_PER_CORE, :]  # (B, Dc, L)
        us = us.reshape(S, SEQ_LEN).T                            # (L, S)
        us = np.ascontiguousarray(us).reshape(NCHUNK, Q, S)
        in_maps.append({"u": us, "tqt": TQt, "pt": PT, "wt": WT, "aq": aq})
    return in_maps


def unshard_output(results):
    """Per-core y shards -> FULL (B, D, L) output."""
    out = np.empty((BATCH, D_MODEL, SEQ_LEN), np.float32)
    for core in range(N_CORES):
        ys = results[core]["y"].reshape(SEQ_LEN, S).T            # (S, L)
        out[:, core * D_PER_CORE:(core + 1) * D_PER_CORE, :] = ys.reshape(
            BATCH, D_PER_CORE, SEQ_LEN
        )
    return out


def kernel(u, A, Bvec, Cvec, L):
    u = np.asarray(u)
    assert u.shape == (BATCH, D_MODEL, SEQ_LEN), u.shape
    nc = get_program()
    in_maps = shard_inputs(u, A, Bvec, Cvec)
    res = run_bass_kernel_spmd(nc, in_maps, list(range(N_CORES)))
    return unshard_output(res.results)


# revision 5
# speedup vs baseline: 1.9168x; 1.9168x over previous
"""Trainium2 Bass kernel for a non-selective (LTI) SSM.

Reference computation (per batch b, channel d):
    h_l = A @ h_{l-1} + Bvec * u[b, d, l]        (h in R^N, A = diag(a))
    y[b, d, l] = Cvec . h_l

Because the system is linear time-invariant and A is diagonal, the scan
collapses into a causal convolution with taps k_j = sum_i C_i a_i^j B_i.
We compute it with a chunked algorithm (chunk length Q = 128):

    y_intra[c] = TQ   @ u[c]      TQ lower-tri Toeplitz from k[0..Q-1]
    s[c]       = P    @ u[c]      end-of-chunk state from in-chunk inputs
    h[c]       = a^Q * h[c-1] + s[c]     (cheap 16-step scan, diagonal)
    y[c]       = y_intra[c] + W @ h[c-1] W[t, i] = C_i a_i^(t+1)

Everything is matmuls on the PE array except the 16-step carry scan.

Sharding: data-parallel over d_model (512 / 8 cores = 64 channels/core);
each core processes S = 4 batches x 64 channels = 256 sequences.
"""

import sys

sys.path.insert(0, "/opt/trn_rl_repo")

import numpy as np

import concourse.bass as bass
import concourse.mybir as mybir
import concourse.tile as tile
from concourse import bacc
from concourse.bass_utils import run_bass_kernel_spmd

N_CORES = 8
BATCH = 4
D_MODEL = 512
SEQ_LEN = 2048
N_STATE = 64
Q = 128                       # chunk length == partition dim
NCHUNK = SEQ_LEN // Q         # 16
D_PER_CORE = D_MODEL // N_CORES  # 64
S = BATCH * D_PER_CORE        # 256 sequences per core
GRP = 4                       # chunks per input DMA group
F32 = mybir.dt.float32
F32R = mybir.dt.float32r      # full-rate fp32 matmul mode (1 cyc/row at N>=256)
DEFAULT_MM_DTYPE = F32R


def build_program(mm_dtype=DEFAULT_MM_DTYPE):
    """Build the per-core Bass program (identical on all 8 cores)."""
    nc = bacc.Bacc(None, target_bir_lowering=False)

    MD = mm_dtype
    u_d = nc.declare_dram_parameter("u", [NCHUNK, Q, S], MD, isOutput=False)
    tqt_d = nc.declare_dram_parameter("tqt", [Q, Q], MD, isOutput=False)
    pt_d = nc.declare_dram_parameter("pt", [Q, N_STATE], MD, isOutput=False)
    wt_d = nc.declare_dram_parameter("wt", [N_STATE, Q], MD, isOutput=False)
    aq_d = nc.declare_dram_parameter("aq", [N_STATE, 1], F32, isOutput=False)
    y_d = nc.declare_dram_parameter("y", [NCHUNK, Q, S], F32, isOutput=True)

    with tile.TileContext(nc) as tc:
        with (
            tc.tile_pool(name="consts", bufs=1) as cpool,
            tc.tile_pool(name="upool", bufs=NCHUNK // GRP) as upool,
            tc.tile_pool(name="hpool", bufs=NCHUNK) as hpool,
            tc.tile_pool(name="ypool", bufs=4) as ypool,
            tc.tile_pool(name="ps_s", bufs=4, space="PSUM") as ps_s,
            tc.tile_pool(name="ps_y", bufs=3, space="PSUM") as ps_y,
        ):
            tqt = cpool.tile([Q, Q], MD)
            nc.sync.dma_start(out=tqt[:], in_=tqt_d[:])
            pt = cpool.tile([Q, N_STATE], MD)
            nc.sync.dma_start(out=pt[:], in_=pt_d[:])
            wt = cpool.tile([N_STATE, Q], MD)
            nc.sync.dma_start(out=wt[:], in_=wt_d[:])
            aq = cpool.tile([N_STATE, 1], F32)
            nc.sync.dma_start(out=aq[:], in_=aq_d[:])

            # u chunks: SBUF layout (t, c_local, s); HBM is (c, t, s)
            u_tiles = []
            for g in range(NCHUNK // GRP):
                ug = upool.tile([Q, GRP, S], MD, name="ug", tag="ug")
                nc.sync.dma_start(
                    out=ug[:],
                    in_=u_d[g * GRP:(g + 1) * GRP].transpose([1, 0, 2]),
                )
                for jj in range(GRP):
                    u_tiles.append(ug[:, jj, :])

            h_prev = None
            for c in range(NCHUNK):
                # end-of-chunk state contribution s[c] = P @ u[c]
                ps = ps_s.tile([N_STATE, S], F32, name="ps", tag="ps")
                nc.tensor.matmul(
                    ps[:], pt[:], u_tiles[c], start=True, stop=True
                )
                # carry scan h[c] = a^Q * h[c-1] + s[c]
                h = hpool.tile([N_STATE, S], MD, name="h", tag="h")
                if c == 0:
                    nc.vector.tensor_copy(out=h[:], in_=ps[:])
                else:
                    nc.vector.scalar_tensor_tensor(
                        out=h[:],
                        in0=h_prev[:],
                        scalar=aq[:],
                        in1=ps[:],
                        op0=mybir.AluOpType.mult,
                        op1=mybir.AluOpType.add,
                    )
                # y[c] = TQ @ u[c] (+ W @ h[c-1])
                py = ps_y.tile([Q, S], F32, name="py", tag="py")
                nc.tensor.matmul(
                    py[:], tqt[:], u_tiles[c], start=True, stop=(c == 0)
                )
                if c > 0:
                    nc.tensor.matmul(
                        py[:], wt[:], h_prev[:], start=False, stop=True
                    )
                yt = ypool.tile([Q, S], F32, name="yt", tag="yt")
                nc.any.tensor_copy(out=yt[:], in_=py[:])
                nc.sync.dma_start(out=y_d[c], in_=yt[:])
                h_prev = h

    nc.compile()
    return nc


def make_params(A, Bvec, Cvec):
    """Host-side precompute of the filter matrices (float64 -> float32)."""
    a = np.diag(np.asarray(A, np.float64))
    B64 = np.asarray(Bvec, np.float64)
    C64 = np.asarray(Cvec, np.float64)
    j = np.arange(Q)
    k = (a[None, :] ** j[:, None]) @ (C64 * B64)        # taps k[0..Q-1]
    TQt = np.zeros((Q, Q), np.float64)                  # TQt[t, jc] = k[jc-t]
    for t in range(Q):
        TQt[t, t:] = k[: Q - t]
    PT = (a[None, :] ** (Q - 1 - j)[:, None]) * B64[None, :]   # (Q, N)
    WT = C64[:, None] * (a[:, None] ** (j[None, :] + 1))       # (N, Q)
    aq = (a ** Q)[:, None]                                      # (N, 1)
    f32c = lambda x: np.ascontiguousarray(x, np.float32)
    return f32c(TQt), f32c(PT), f32c(WT), f32c(aq)


_prog_cache = {}


def get_program(mm_dtype=DEFAULT_MM_DTYPE):
    key = str(mm_dtype)
    if key not in _prog_cache:
        _prog_cache[key] = build_program(mm_dtype)
    return _prog_cache[key]


def shard_inputs(u, A, Bvec, Cvec):
    """FULL inputs -> per-core in_maps."""
    TQt, PT, WT, aq = make_params(A, Bvec, Cvec)
    u = np.asarray(u, np.float32)
    in_maps = []
    for core in range(N_CORES):
        us = u[:, core * D_PER_CORE:(core + 1) * D_PER_CORE, :]  # (B, Dc, L)
        us = us.reshape(S, SEQ_LEN).T                            # (L, S)
        us = np.ascontiguousarray(us).reshape(NCHUNK, Q, S)
        in_maps.append({"u": us, "tqt": TQt, "pt": PT, "wt": WT, "aq": aq})
    return in_maps


def unshard_output(results):
    """Per-core y shards -> FULL (B, D, L) output."""
    out = np.empty((BATCH, D_MODEL, SEQ_LEN), np.float32)
    for core in range(N_CORES):
        ys = results[core]["y"].reshape(SEQ_LEN, S).T            # (S, L)
        out[:, core * D_PER_CORE:(core + 1) * D_PER_CORE, :] = ys.reshape(
            BATCH, D_PER_CORE, SEQ_LEN
        )
    return out


def kernel(u, A, Bvec, Cvec, L):
    u = np.asarray(u)
    assert u.shape == (BATCH, D_MODEL, SEQ_LEN), u.shape
    nc = get_program()
    in_maps = shard_inputs(u, A, Bvec, Cvec)
    res = run_bass_kernel_spmd(nc, in_maps, list(range(N_CORES)))
    return unshard_output(res.results)


# revision 7
# speedup vs baseline: 1.9685x; 1.0270x over previous
"""Trainium2 Bass kernel for a non-selective (LTI) SSM.

Reference computation (per batch b, channel d):
    h_l = A @ h_{l-1} + Bvec * u[b, d, l]        (h in R^N, A = diag(a))
    y[b, d, l] = Cvec . h_l

Because the system is linear time-invariant and A is diagonal, the scan
collapses into a causal convolution with taps k_j = sum_i C_i a_i^j B_i.
We compute it with a chunked algorithm (chunk length Q = 128):

    y_intra[c] = TQ   @ u[c]      TQ lower-tri Toeplitz from k[0..Q-1]
    s[c]       = P    @ u[c]      end-of-chunk state from in-chunk inputs
    h[c]       = a^Q * h[c-1] + s[c]     (cheap 16-step scan, diagonal)
    y[c]       = y_intra[c] + W @ h[c-1] W[t, i] = C_i a_i^(t+1)

Everything is matmuls on the PE array except the 16-step carry scan.

Sharding: data-parallel over d_model (512 / 8 cores = 64 channels/core);
each core processes S = 4 batches x 64 channels = 256 sequences.
"""

import sys

sys.path.insert(0, "/opt/trn_rl_repo")

import numpy as np

import concourse.bass as bass
import concourse.mybir as mybir
import concourse.tile as tile
from concourse import bacc
from concourse.bass_utils import run_bass_kernel_spmd

N_CORES = 8
BATCH = 4
D_MODEL = 512
SEQ_LEN = 2048
N_STATE = 64
Q = 128                       # chunk length == partition dim
NCHUNK = SEQ_LEN // Q         # 16
D_PER_CORE = D_MODEL // N_CORES  # 64
S = BATCH * D_PER_CORE        # 256 sequences per core
GRP = 4                       # chunks per input DMA group
F32 = mybir.dt.float32
F32R = mybir.dt.float32r      # single-instruction fp32 matmul (2x fp32 tput)
DEFAULT_MM_DTYPE = F32R
N_WARMUP = 10                 # dummy matmuls to lift the PE HAM clock gate

# packed const columns: [TQt | PT | WT]
C_TQT, C_PT, C_WT = 0, Q, Q + N_STATE
C_TOT = Q + N_STATE + Q       # 320


def build_program(mm_dtype=DEFAULT_MM_DTYPE):
    """Build the per-core Bass program (identical on all 8 cores)."""
    nc = bacc.Bacc(None, target_bir_lowering=False)

    MD = mm_dtype
    u_d = nc.declare_dram_parameter("u", [NCHUNK, Q, S], MD, isOutput=False)
    cs_d = nc.declare_dram_parameter("consts", [Q, C_TOT], MD, isOutput=False)
    aq_d = nc.declare_dram_parameter("aq", [N_STATE, 1], F32, isOutput=False)
    y_d = nc.declare_dram_parameter("y", [NCHUNK, Q, S], F32, isOutput=True)

    with tile.TileContext(nc) as tc:
        with (
            tc.tile_pool(name="warm", bufs=1) as wpool,
            tc.tile_pool(name="consts", bufs=1) as cpool,
            tc.tile_pool(name="upool", bufs=NCHUNK // GRP) as upool,
            tc.tile_pool(name="hpool", bufs=NCHUNK) as hpool,
            tc.tile_pool(name="ypool", bufs=4) as ypool,
            tc.tile_pool(name="ps_warm", bufs=1, space="PSUM") as ps_w,
            tc.tile_pool(name="ps_s", bufs=4, space="PSUM") as ps_s,
            tc.tile_pool(name="ps_y", bufs=3, space="PSUM") as ps_y,
        ):
            # ---- PE warm-up: dummy matmuls on zeroed scratch, no data deps.
            # They run during the initial DMA window and lift the HAM clock
            # gate (1.2 -> 2.4 GHz) before the real matmuls start.
            wsrc = wpool.tile([Q, 512], mybir.dt.bfloat16)
            nc.gpsimd.memset(wsrc[:], 0.0)
            wps = ps_w.tile([Q, 512], F32)
            for _ in range(N_WARMUP):
                nc.tensor.matmul(wps[:], wsrc[:, :Q], wsrc[:], start=True,
                                 stop=True)

            # ---- input DMAs (sync queue): first u group first, then consts
            u_tiles = []
            ug_tiles = []
            for g in range(NCHUNK // GRP):
                ug = upool.tile([Q, GRP, S], MD, name="ug", tag="ug")
                ug_tiles.append(ug)
            nc.sync.dma_start(
                out=ug_tiles[0][:], in_=u_d[0:GRP].transpose([1, 0, 2])
            )
            cs = cpool.tile([Q, C_TOT], MD)
            nc.sync.dma_start(out=cs[:], in_=cs_d[:])
            for g in range(1, NCHUNK // GRP):
                nc.sync.dma_start(
                    out=ug_tiles[g][:],
                    in_=u_d[g * GRP:(g + 1) * GRP].transpose([1, 0, 2]),
                )
            for g in range(NCHUNK // GRP):
                for jj in range(GRP):
                    u_tiles.append(ug_tiles[g][:, jj, :])
            # aq on the gpsimd (SWDGE) queue; needed only by the carry scan
            aq = cpool.tile([N_STATE, 1], F32)
            nc.gpsimd.dma_start(out=aq[:], in_=aq_d[:])

            tqt = cs[:, C_TQT:C_TQT + Q]
            pt = cs[:, C_PT:C_PT + N_STATE]
            wt = cs[:N_STATE, C_WT:C_WT + Q]

            h_prev = None
            for c in range(NCHUNK):
                # end-of-chunk state contribution s[c] = P @ u[c]
                ps = ps_s.tile([N_STATE, S], F32, name="ps", tag="ps")
                nc.tensor.matmul(ps[:], pt, u_tiles[c], start=True, stop=True)
                # carry scan h[c] = a^Q * h[c-1] + s[c]
                h = hpool.tile([N_STATE, S], MD, name="h", tag="h")
                if c == 0:
                    nc.vector.tensor_copy(out=h[:], in_=ps[:])
                else:
                    nc.vector.scalar_tensor_tensor(
                        out=h[:],
                        in0=h_prev[:],
                        scalar=aq[:],
                        in1=ps[:],
                        op0=mybir.AluOpType.mult,
                        op1=mybir.AluOpType.add,
                    )
                # y[c] = TQ @ u[c] (+ W @ h[c-1])
                py = ps_y.tile([Q, S], F32, name="py", tag="py")
                nc.tensor.matmul(
                    py[:], tqt, u_tiles[c], start=True, stop=(c == 0)
                )
                if c > 0:
                    nc.tensor.matmul(
                        py[:], wt, h_prev[:], start=False, stop=True
                    )
                yt = ypool.tile([Q, S], F32, name="yt", tag="yt")
                nc.vector.tensor_copy(out=yt[:], in_=py[:])
                # output DMAs on the scalar HWDGE queue (parallel to sync's)
                nc.scalar.dma_start(out=y_d[c], in_=yt[:])
                h_prev = h

    nc.compile()
    return nc


def make_params(A, Bvec, Cvec):
    """Host-side precompute of the filter matrices (float64 -> float32)."""
    a = np.diag(np.asarray(A, np.float64))
    B64 = np.asarray(Bvec, np.float64)
    C64 = np.asarray(Cvec, np.float64)
    j = np.arange(Q)
    k = (a[None, :] ** j[:, None]) @ (C64 * B64)        # taps k[0..Q-1]
    TQt = np.zeros((Q, Q), np.float64)                  # TQt[t, jc] = k[jc-t]
    for t in range(Q):
        TQt[t, t:] = k[: Q - t]
    PT = (a[None, :] ** (Q - 1 - j)[:, None]) * B64[None, :]   # (Q, N)
    WT = C64[:, None] * (a[:, None] ** (j[None, :] + 1))       # (N, Q)
    aq = (a ** Q)[:, None]                                      # (N, 1)
    consts = np.zeros((Q, C_TOT), np.float64)
    consts[:, C_TQT:C_TQT + Q] = TQt
    consts[:, C_PT:C_PT + N_STATE] = PT
    consts[:N_STATE, C_WT:C_WT + Q] = WT
    f32c = lambda x: np.ascontiguousarray(x, np.float32)
    return f32c(consts), f32c(aq)


_prog_cache = {}


def get_program(mm_dtype=DEFAULT_MM_DTYPE):
    key = str(mm_dtype)
    if key not in _prog_cache:
        _prog_cache[key] = build_program(mm_dtype)
    return _prog_cache[key]


def shard_inputs(u, A, Bvec, Cvec):
    """FULL inputs -> per-core in_maps."""
    consts, aq = make_params(A, Bvec, Cvec)
    u = np.asarray(u, np.float32)
    in_maps = []
    for core in range(N_CORES):
        us = u[:, core * D_PER_CORE:(core + 1) * D_PER_CORE, :]  # (B, Dc, L)
        us = us.reshape(S, SEQ_LEN).T                            # (L, S)
        us = np.ascontiguousarray(us).reshape(NCHUNK, Q, S)
        in_maps.append({"u": us, "consts": consts, "aq": aq})
    return in_maps


def unshard_output(results):
    """Per-core y shards -> FULL (B, D, L) output."""
    out = np.empty((BATCH, D_MODEL, SEQ_LEN), np.float32)
    for core in range(N_CORES):
        ys = results[core]["y"].reshape(SEQ_LEN, S).T            # (S, L)
        out[:, core * D_PER_CORE:(core + 1) * D_PER_CORE, :] = ys.reshape(
            BATCH, D_PER_CORE, SEQ_LEN
        )
    return out


def kernel(u, A, Bvec, Cvec, L):
    u = np.asarray(u)
    assert u.shape == (BATCH, D_MODEL, SEQ_LEN), u.shape
    nc = get_program()
    in_maps = shard_inputs(u, A, Bvec, Cvec)
    res = run_bass_kernel_spmd(nc, in_maps, list(range(N_CORES)))
    return unshard_output(res.results)


# revision 8
# speedup vs baseline: 2.0179x; 1.0251x over previous
"""Trainium2 Bass kernel for a non-selective (LTI) SSM.

Reference computation (per batch b, channel d):
    h_l = A @ h_{l-1} + Bvec * u[b, d, l]        (h in R^N, A = diag(a))
    y[b, d, l] = Cvec . h_l

Because the system is linear time-invariant and A is diagonal, the scan
collapses into a causal convolution with taps k_j = sum_i C_i a_i^j B_i.
We compute it with a chunked algorithm (chunk length Q = 128):

    y_intra[c] = TQ   @ u[c]      TQ lower-tri Toeplitz from k[0..Q-1]
    s[c]       = P    @ u[c]      end-of-chunk state from in-chunk inputs
    h[c]       = a^Q * h[c-1] + s[c]     (cheap 16-step scan, diagonal)
    y[c]       = y_intra[c] + W @ h[c-1] W[t, i] = C_i a_i^(t+1)

Everything is matmuls on the PE array except the 16-step carry scan.

Sharding: data-parallel over d_model (512 / 8 cores = 64 channels/core);
each core processes S = 4 batches x 64 channels = 256 sequences.
"""

import sys

sys.path.insert(0, "/opt/trn_rl_repo")

import numpy as np

import concourse.bass as bass
import concourse.mybir as mybir
import concourse.tile as tile
from concourse import bacc
from concourse.bass_utils import run_bass_kernel_spmd

N_CORES = 8
BATCH = 4
D_MODEL = 512
SEQ_LEN = 2048
N_STATE = 64
Q = 128                       # chunk length == partition dim
NCHUNK = SEQ_LEN // Q         # 16
D_PER_CORE = D_MODEL // N_CORES  # 64
S = BATCH * D_PER_CORE        # 256 sequences per core
GRP = 4                       # chunks per input DMA group
F32 = mybir.dt.float32
F32R = mybir.dt.float32r      # single-instruction fp32 matmul (2x fp32 tput)
DEFAULT_MM_DTYPE = F32R
N_WARMUP = 6                 # dummy matmuls to lift the PE HAM clock gate

# packed const columns: [TQt | PT | WT]
C_TQT, C_PT, C_WT = 0, Q, Q + N_STATE
C_TOT = Q + N_STATE + Q       # 320


def build_program(mm_dtype=DEFAULT_MM_DTYPE):
    """Build the per-core Bass program (identical on all 8 cores)."""
    nc = bacc.Bacc(None, target_bir_lowering=False)

    MD = mm_dtype
    u_d = nc.declare_dram_parameter("u", [NCHUNK, Q, S], MD, isOutput=False)
    cs_d = nc.declare_dram_parameter("consts", [Q, C_TOT], MD, isOutput=False)
    aq_d = nc.declare_dram_parameter("aq", [N_STATE, 1], F32, isOutput=False)
    y_d = nc.declare_dram_parameter("y", [NCHUNK, Q, S], F32, isOutput=True)

    with tile.TileContext(nc) as tc:
        with (
            tc.tile_pool(name="warm", bufs=1) as wpool,
            tc.tile_pool(name="consts", bufs=1) as cpool,
            tc.tile_pool(name="upool", bufs=NCHUNK // GRP) as upool,
            tc.tile_pool(name="hpool", bufs=NCHUNK) as hpool,
            tc.tile_pool(name="ypool", bufs=4) as ypool,
            tc.tile_pool(name="ps_warm", bufs=1, space="PSUM") as ps_w,
            tc.tile_pool(name="ps_s", bufs=3, space="PSUM") as ps_s,
            tc.tile_pool(name="ps_y", bufs=4, space="PSUM") as ps_y,
        ):
            # ---- PE warm-up: dummy matmuls on zeroed scratch, no data deps.
            # They run during the initial DMA window and lift the HAM clock
            # gate (1.2 -> 2.4 GHz) before the real matmuls start.
            wsrc = wpool.tile([Q, 512], mybir.dt.bfloat16)
            nc.gpsimd.memset(wsrc[:], 0.0)
            wps = ps_w.tile([Q, 512], F32)
            for _ in range(N_WARMUP):
                nc.tensor.matmul(wps[:], wsrc[:, :Q], wsrc[:], start=True,
                                 stop=True)

            # ---- input DMAs (sync queue): consts, then u chunk 0 alone (so
            # compute can start as early as possible), then the rest.
            cs = cpool.tile([Q, C_TOT], MD)
            nc.sync.dma_start(out=cs[:], in_=cs_d[:])
            aq = cpool.tile([N_STATE, 1], F32)
            u_tiles = []
            ug_tiles = []
            for g in range(NCHUNK // GRP):
                ug = upool.tile([Q, GRP, S], MD, name="ug", tag="ug")
                ug_tiles.append(ug)
            nc.sync.dma_start(
                out=ug_tiles[0][:, 0, :], in_=u_d[0].transpose([0, 1])
            )
            nc.sync.dma_start(out=aq[:], in_=aq_d[:])
            nc.sync.dma_start(
                out=ug_tiles[0][:, 1:GRP, :],
                in_=u_d[1:GRP].transpose([1, 0, 2]),
            )
            for g in range(1, NCHUNK // GRP):
                nc.sync.dma_start(
                    out=ug_tiles[g][:],
                    in_=u_d[g * GRP:(g + 1) * GRP].transpose([1, 0, 2]),
                )
            for g in range(NCHUNK // GRP):
                for jj in range(GRP):
                    u_tiles.append(ug_tiles[g][:, jj, :])

            tqt = cs[:, C_TQT:C_TQT + Q]
            pt = cs[:, C_PT:C_PT + N_STATE]
            wt = cs[:N_STATE, C_WT:C_WT + Q]

            h_prev = None
            for c in range(NCHUNK):
                # end-of-chunk state contribution s[c] = P @ u[c]
                ps = ps_s.tile([N_STATE, S], F32, name="ps", tag="ps")
                nc.tensor.matmul(ps[:], pt, u_tiles[c], start=True, stop=True)
                # carry scan h[c] = a^Q * h[c-1] + s[c]
                h = hpool.tile([N_STATE, S], MD, name="h", tag="h")
                if c == 0:
                    nc.vector.tensor_copy(out=h[:], in_=ps[:])
                else:
                    nc.vector.scalar_tensor_tensor(
                        out=h[:],
                        in0=h_prev[:],
                        scalar=aq[:],
                        in1=ps[:],
                        op0=mybir.AluOpType.mult,
                        op1=mybir.AluOpType.add,
                    )
                # y[c] = TQ @ u[c] (+ W @ h[c-1])
                py = ps_y.tile([Q, S], F32, name="py", tag="py")
                nc.tensor.matmul(
                    py[:], tqt, u_tiles[c], start=True, stop=(c == 0)
                )
                if c > 0:
                    nc.tensor.matmul(
                        py[:], wt, h_prev[:], start=False, stop=True
                    )
                yt = ypool.tile([Q, S], F32, name="yt", tag="yt")
                # PSUM->SBUF eviction on ScalarE; DVE is busy with the scan
                nc.scalar.copy(out=yt[:], in_=py[:])
                # output DMAs on the gpsimd SWDGE queue (GpSimd is idle)
                nc.gpsimd.dma_start(out=y_d[c], in_=yt[:])
                h_prev = h

    nc.compile()
    return nc


def make_params(A, Bvec, Cvec):
    """Host-side precompute of the filter matrices (float64 -> float32)."""
    a = np.diag(np.asarray(A, np.float64))
    B64 = np.asarray(Bvec, np.float64)
    C64 = np.asarray(Cvec, np.float64)
    j = np.arange(Q)
    k = (a[None, :] ** j[:, None]) @ (C64 * B64)        # taps k[0..Q-1]
    TQt = np.zeros((Q, Q), np.float64)                  # TQt[t, jc] = k[jc-t]
    for t in range(Q):
        TQt[t, t:] = k[: Q - t]
    PT = (a[None, :] ** (Q - 1 - j)[:, None]) * B64[None, :]   # (Q, N)
    WT = C64[:, None] * (a[:, None] ** (j[None, :] + 1))       # (N, Q)
    aq = (a ** Q)[:, None]                                      # (N, 1)
    consts = np.zeros((Q, C_TOT), np.float64)
    consts[:, C_TQT:C_TQT + Q] = TQt
    consts[:, C_PT:C_PT + N_STATE] = PT
    consts[:N_STATE, C_WT:C_WT + Q] = WT
    f32c = lambda x: np.ascontiguousarray(x, np.float32)
    return f32c(consts), f32c(aq)


_prog_cache = {}


def get_program(mm_dtype=DEFAULT_MM_DTYPE):
    key = str(mm_dtype)
    if key not in _prog_cache:
        _prog_cache[key] = build_program(mm_dtype)
    return _prog_cache[key]


def shard_inputs(u, A, Bvec, Cvec):
    """FULL inputs -> per-core in_maps."""
    consts, aq = make_params(A, Bvec, Cvec)
    u = np.asarray(u, np.float32)
    in_maps = []
    for core in range(N_CORES):
        us = u[:, core * D_PER_CORE:(core + 1) * D_PER_CORE, :]  # (B, Dc, L)
        us = us.reshape(S, SEQ_LEN).T                            # (L, S)
        us = np.ascontiguousarray(us).reshape(NCHUNK, Q, S)
        in_maps.append({"u": us, "consts": consts, "aq": aq})
    return in_maps


def unshard_output(results):
    """Per-core y shards -> FULL (B, D, L) output."""
    out = np.empty((BATCH, D_MODEL, SEQ_LEN), np.float32)
    for core in range(N_CORES):
        ys = results[core]["y"].reshape(SEQ_LEN, S).T            # (S, L)
        out[:, core * D_PER_CORE:(core + 1) * D_PER_CORE, :] = ys.reshape(
            BATCH, D_PER_CORE, SEQ_LEN
        )
    return out


def kernel(u, A, Bvec, Cvec, L):
    u = np.asarray(u)
    assert u.shape == (BATCH, D_MODEL, SEQ_LEN), u.shape
    nc = get_program()
    in_maps = shard_inputs(u, A, Bvec, Cvec)
    res = run_bass_kernel_spmd(nc, in_maps, list(range(N_CORES)))
    return unshard_output(res.results)


# revision 9
# speedup vs baseline: 2.0328x; 1.0074x over previous
"""Trainium2 Bass kernel for a non-selective (LTI) SSM.

Reference computation (per batch b, channel d):
    h_l = A @ h_{l-1} + Bvec * u[b, d, l]        (h in R^N, A = diag(a))
    y[b, d, l] = Cvec . h_l

Because the system is linear time-invariant and A is diagonal, the scan
collapses into a causal convolution with taps k_j = sum_i C_i a_i^j B_i.
We compute it with a chunked algorithm (chunk length Q = 128):

    y_intra[c] = TQ   @ u[c]      TQ lower-tri Toeplitz from k[0..Q-1]
    s[c]       = P    @ u[c]      end-of-chunk state from in-chunk inputs
    h[c]       = a^Q * h[c-1] + s[c]     (cheap 16-step scan, diagonal)
    y[c]       = y_intra[c] + W @ h[c-1] W[t, i] = C_i a_i^(t+1)

Everything is matmuls on the PE array except the 16-step carry scan.

Sharding: data-parallel over d_model (512 / 8 cores = 64 channels/core);
each core processes S = 4 batches x 64 channels = 256 sequences.
"""

import sys

sys.path.insert(0, "/opt/trn_rl_repo")

import numpy as np

import concourse.bass as bass
import concourse.mybir as mybir
import concourse.tile as tile
from concourse import bacc
from concourse.bass_utils import run_bass_kernel_spmd

N_CORES = 8
BATCH = 4
D_MODEL = 512
SEQ_LEN = 2048
N_STATE = 64
Q = 128                       # chunk length == partition dim
NCHUNK = SEQ_LEN // Q         # 16
D_PER_CORE = D_MODEL // N_CORES  # 64
S = BATCH * D_PER_CORE        # 256 sequences per core
GRP = 4                       # chunks per input DMA group
F32 = mybir.dt.float32
F32R = mybir.dt.float32r      # single-instruction fp32 matmul (2x fp32 tput)
DEFAULT_MM_DTYPE = F32R
N_WARMUP = 6                 # dummy matmuls to lift the PE HAM clock gate

# packed const columns: [TQt | PT | WT]
C_TQT, C_PT, C_WT = 0, Q, Q + N_STATE
C_TOT = Q + N_STATE + Q       # 320


def build_program(mm_dtype=DEFAULT_MM_DTYPE):
    """Build the per-core Bass program (identical on all 8 cores)."""
    nc = bacc.Bacc(None, target_bir_lowering=False)

    MD = mm_dtype
    u_d = nc.declare_dram_parameter("u", [NCHUNK, Q, S], MD, isOutput=False)
    cs_d = nc.declare_dram_parameter("consts", [Q, C_TOT], MD, isOutput=False)
    aq_d = nc.declare_dram_parameter("aq", [N_STATE, 1], F32, isOutput=False)
    y_d = nc.declare_dram_parameter("y", [NCHUNK, Q, S], F32, isOutput=True)

    with tile.TileContext(nc) as tc:
        with (
            tc.tile_pool(name="warm", bufs=1) as wpool,
            tc.tile_pool(name="consts", bufs=1) as cpool,
            tc.tile_pool(name="upool", bufs=NCHUNK // GRP) as upool,
            tc.tile_pool(name="hpool", bufs=NCHUNK) as hpool,
            tc.tile_pool(name="ypool", bufs=4) as ypool,
            tc.tile_pool(name="ps_warm", bufs=1, space="PSUM") as ps_w,
            tc.tile_pool(name="ps_s", bufs=3, space="PSUM") as ps_s,
            tc.tile_pool(name="ps_y", bufs=4, space="PSUM") as ps_y,
        ):
            # ---- PE warm-up: dummy matmuls on zeroed scratch, no data deps.
            # They run during the initial DMA window and lift the HAM clock
            # gate (1.2 -> 2.4 GHz) before the real matmuls start.
            wsrc = wpool.tile([Q, 512], mybir.dt.bfloat16)
            nc.gpsimd.memset(wsrc[:], 0.0)
            wps = ps_w.tile([Q, 512], F32)
            for _ in range(N_WARMUP):
                nc.tensor.matmul(wps[:], wsrc[:, :Q], wsrc[:], start=True,
                                 stop=True)

            # ---- input DMAs (sync queue): consts, then u chunk 0 alone (so
            # compute can start as early as possible), then the rest.
            cs = cpool.tile([Q, C_TOT], MD)
            nc.sync.dma_start(out=cs[:], in_=cs_d[:])
            aq = cpool.tile([N_STATE, 1], F32)
            u_tiles = []
            ug_tiles = []
            for g in range(NCHUNK // GRP):
                ug = upool.tile([Q, GRP, S], MD, name="ug", tag="ug")
                ug_tiles.append(ug)
            nc.sync.dma_start(
                out=ug_tiles[0][:, 0, :], in_=u_d[0].transpose([0, 1])
            )
            nc.sync.dma_start(out=aq[:], in_=aq_d[:])
            nc.sync.dma_start(
                out=ug_tiles[0][:, 1:GRP, :],
                in_=u_d[1:GRP].transpose([1, 0, 2]),
            )
            for g in range(1, NCHUNK // GRP):
                nc.sync.dma_start(
                    out=ug_tiles[g][:],
                    in_=u_d[g * GRP:(g + 1) * GRP].transpose([1, 0, 2]),
                )
            for g in range(NCHUNK // GRP):
                for jj in range(GRP):
                    u_tiles.append(ug_tiles[g][:, jj, :])

            tqt = cs[:, C_TQT:C_TQT + Q]
            pt = cs[:, C_PT:C_PT + N_STATE]
            wt = cs[:N_STATE, C_WT:C_WT + Q]

            h_prev = None
            for c in range(NCHUNK):
                # y_intra first: its PSUM drain overlaps the s matmul below,
                # so the accumulating inter matmul doesn't stall on the bank.
                py = ps_y.tile([Q, S], F32, name="py", tag="py")
                nc.tensor.matmul(
                    py[:], tqt, u_tiles[c], start=True, stop=(c == 0)
                )
                # end-of-chunk state contribution s[c] = P @ u[c]
                ps = ps_s.tile([N_STATE, S], F32, name="ps", tag="ps")
                nc.tensor.matmul(ps[:], pt, u_tiles[c], start=True, stop=True)
                # y[c] += W @ h[c-1]
                if c > 0:
                    nc.tensor.matmul(
                        py[:], wt, h_prev[:], start=False, stop=True
                    )
                # carry scan h[c] = a^Q * h[c-1] + s[c]
                h = hpool.tile([N_STATE, S], MD, name="h", tag="h")
                if c == 0:
                    nc.vector.tensor_copy(out=h[:], in_=ps[:])
                else:
                    nc.vector.scalar_tensor_tensor(
                        out=h[:],
                        in0=h_prev[:],
                        scalar=aq[:],
                        in1=ps[:],
                        op0=mybir.AluOpType.mult,
                        op1=mybir.AluOpType.add,
                    )
                yt = ypool.tile([Q, S], F32, name="yt", tag="yt")
                # PSUM->SBUF eviction on ScalarE; DVE is busy with the scan
                nc.scalar.copy(out=yt[:], in_=py[:])
                # output DMAs on the gpsimd SWDGE queue (GpSimd is idle)
                nc.gpsimd.dma_start(out=y_d[c], in_=yt[:])
                h_prev = h

    nc.compile()
    return nc


def make_params(A, Bvec, Cvec):
    """Host-side precompute of the filter matrices (float64 -> float32)."""
    a = np.diag(np.asarray(A, np.float64))
    B64 = np.asarray(Bvec, np.float64)
    C64 = np.asarray(Cvec, np.float64)
    j = np.arange(Q)
    k = (a[None, :] ** j[:, None]) @ (C64 * B64)        # taps k[0..Q-1]
    TQt = np.zeros((Q, Q), np.float64)                  # TQt[t, jc] = k[jc-t]
    for t in range(Q):
        TQt[t, t:] = k[: Q - t]
    PT = (a[None, :] ** (Q - 1 - j)[:, None]) * B64[None, :]   # (Q, N)
    WT = C64[:, None] * (a[:, None] ** (j[None, :] + 1))       # (N, Q)
    aq = (a ** Q)[:, None]                                      # (N, 1)
    consts = np.zeros((Q, C_TOT), np.float64)
    consts[:, C_TQT:C_TQT + Q] = TQt
    consts[:, C_PT:C_PT + N_STATE] = PT
    consts[:N_STATE, C_WT:C_WT + Q] = WT
    f32c = lambda x: np.ascontiguousarray(x, np.float32)
    return f32c(consts), f32c(aq)


_prog_cache = {}


def get_program(mm_dtype=DEFAULT_MM_DTYPE):
    key = str(mm_dtype)
    if key not in _prog_cache:
        _prog_cache[key] = build_program(mm_dtype)
    return _prog_cache[key]


def shard_inputs(u, A, Bvec, Cvec):
    """FULL inputs -> per-core in_maps."""
    consts, aq = make_params(A, Bvec, Cvec)
    u = np.asarray(u, np.float32)
    in_maps = []
    for core in range(N_CORES):
        us = u[:, core * D_PER_CORE:(core + 1) * D_PER_CORE, :]  # (B, Dc, L)
        us = us.reshape(S, SEQ_LEN).T                            # (L, S)
        us = np.ascontiguousarray(us).reshape(NCHUNK, Q, S)
        in_maps.append({"u": us, "consts": consts, "aq": aq})
    return in_maps


def unshard_output(results):
    """Per-core y shards -> FULL (B, D, L) output."""
    out = np.empty((BATCH, D_MODEL, SEQ_LEN), np.float32)
    for core in range(N_CORES):
        ys = results[core]["y"].reshape(SEQ_LEN, S).T            # (S, L)
        out[:, core * D_PER_CORE:(core + 1) * D_PER_CORE, :] = ys.reshape(
            BATCH, D_PER_CORE, SEQ_LEN
        )
    return out


def kernel(u, A, Bvec, Cvec, L):
    u = np.asarray(u)
    assert u.shape == (BATCH, D_MODEL, SEQ_LEN), u.shape
    nc = get_program()
    in_maps = shard_inputs(u, A, Bvec, Cvec)
    res = run_bass_kernel_spmd(nc, in_maps, list(range(N_CORES)))
    return unshard_output(res.results)
